# revision 1
# baseline (speedup 1.0000x reference)
"""GAT layer kernel for 8 Trainium2 NeuronCores.

Strategy (row-sharded attention, per the sharding hint):
  - Core c owns query rows [c*1024, (c+1)*1024) of the 8000-node graph
    (1024 = 8*128; core 7's slice is host-padded with zero rows; the key
    axis is padded to 8064 = 63*128 with zero adjacency columns).
  - Shard layout choice: each core's adjacency slice is uploaded
    TRANSPOSED ([8064 keys, 1024 queries] int32) so the attention matrix
    is built directly with keys on partitions — the layout the TensorE
    contraction needs. The kernel still streams the same 32 MB of int32
    adjacency per core from HBM (the memory-bound term is unchanged);
    SWDGE casts it to {0,1} fp16 in flight.
  - H = X @ W_w^T is computed replicated on every core from a host-
    transposed bf16 X. W_b is folded into the final output (softmax rows
    sum to 1 => attn @ (H0 + 1 W_b^T) = attn@H0 + W_b); its contribution
    to the scores goes through host-folded constants c_src/c_dst.
  - Scores in transposed layout: e = s_src[i] (broadcast tile, built by
    a rank-1 PE matmul) + s_dst[j] (per-partition scalar) on VectorE;
    leakyrelu as (0.2*e) max e in one scalar_tensor_tensor; exp on
    ScalarE (no max-subtraction: |e| <~ 3 so exp is safe); mask applied
    after exp (mask*exp(lrelu(e)) is exactly the reference's
    mask->-inf->lrelu->softmax weights) as one GpSimd multiply.
  - attn_unnorm @ [H | 1] runs as fp16 matmuls accumulating into 8 PSUM
    banks (one per 126-row output block); column 256 gives the softmax
    denominator, applied as a reciprocal per-partition multiply (plus
    the W_b add) while draining PSUM.
"""
import sys

sys.path.insert(0, "/opt/trn_rl_repo")

import numpy as np
import ml_dtypes

N, F = 8000, 256
NP = 8064          # padded key count (63 * 128)
W = 1024           # query rows per core (8 * 128; last core mostly padding)
NJT = NP // 128    # 63 key tiles
MB = W // 8        # 128-row output blocks (full-width weights enable FWL)
NEG_SLOPE = 0.2

_RUNNER = None
_last_in_maps = None


def _build(repeat=1):
    import concourse.bass as bass
    import concourse.tile as tile
    from concourse import bacc, mybir

    f16 = mybir.dt.float16
    f32 = mybir.dt.float32
    bf16 = mybir.dt.bfloat16

    nc = bacc.Bacc()
    adjtd = nc.dram_tensor("adjt", (NP, W), mybir.dt.int32, kind="ExternalInput")
    xtd = nc.dram_tensor("xtb", (F, NP), bf16, kind="ExternalInput")
    xwtd = nc.dram_tensor("xwtb", (F, W), bf16, kind="ExternalInput")
    wgd = nc.dram_tensor("wg", (F, 258), bf16, kind="ExternalInput")
    wbd = nc.dram_tensor("wbt", (128, F), f16, kind="ExternalInput")
    cvd = nc.dram_tensor("cv", (128, 2), f32, kind="ExternalInput")
    outd = nc.dram_tensor("out", (W, F), f32, kind="ExternalOutput")

    with tile.TileContext(nc) as tc:
        with (
            tc.tile_pool(name="pp", bufs=1) as pp,
            tc.tile_pool(name="att", bufs=2) as ap_,
            tc.tile_pool(name="fin", bufs=2) as fin,
            tc.tile_pool(name="ps", bufs=8, space="PSUM") as psp,
        ):
            for _rep in range(repeat):
                # ---- phase 0a: parameters and transposed activations ----
                wg_sb = [pp.tile([128, 258], bf16, name=f"wg{k}", tag=f"wg{k}") for k in range(2)]
                xt = [pp.tile([128, NP], bf16, name=f"xt{k}", tag=f"xt{k}") for k in range(2)]
                xwt = [pp.tile([128, W], bf16, name=f"xwt{k}", tag=f"xwt{k}") for k in range(2)]
                for k in range(2):
                    nc.sync.dma_start(wg_sb[k][:], wgd[k * 128 : (k + 1) * 128, :])
                    nc.sync.dma_start(xt[k][:], xtd[k * 128 : (k + 1) * 128, :])
                    nc.sync.dma_start(xwt[k][:], xwtd[k * 128 : (k + 1) * 128, :])
                wb_sb = pp.tile([128, F], f16)
                nc.sync.dma_start(wb_sb[:], wbd[:])
                cv_sb = pp.tile([128, 2], f32)
                nc.sync.dma_start(cv_sb[:], cvd[:])

                # ---- phase 0b: s_src row (this core's queries) ----
                ssrc_row = pp.tile([1, W], f32)
                for ch in range(2):
                    ps = psp.tile([2, 512], f32, name="ps_s", tag="ps")
                    for k in range(2):
                        nc.tensor.matmul(
                            ps[:],
                            wg_sb[k][:, 256:258],
                            xwt[k][:, ch * 512 : (ch + 1) * 512],
                            start=(k == 0),
                            stop=(k == 1),
                        )
                    nc.vector.tensor_scalar_add(
                        ssrc_row[0:1, ch * 512 : (ch + 1) * 512], ps[0:1, :], cv_sb[0:1, 0:1]
                    )

                # ---- phase 0c: broadcast s_src across partitions via PE ----
                ones1 = pp.tile([1, 128], f32)
                nc.vector.memset(ones1[:], 1.0)
                sb1 = pp.tile([128, W], f16)
                for ch in range(2):
                    psb_t = psp.tile([128, 512], f32, name="ps_b", tag="ps")
                    nc.tensor.matmul(
                        psb_t[:], ones1[:], ssrc_row[0:1, ch * 512 : (ch + 1) * 512],
                        start=True, stop=True,
                    )
                    nc.vector.tensor_copy(sb1[:, ch * 512 : (ch + 1) * 512], psb_t[:])

                # ---- phase 0d: H' key tiles + s_dst columns ----
                hp = pp.tile([128, NJT * 257], f16)
                sdst = pp.tile([128, NJT], f32)
                for jt in range(NJT):
                    ph = psp.tile([128, 258], f32, name="ps_h", tag="ps")
                    for k in range(2):
                        nc.tensor.matmul(
                            ph[:],
                            xt[k][:, jt * 128 : (jt + 1) * 128],
                            wg_sb[k][:],
                            start=(k == 0),
                            stop=(k == 1),
                        )
                    if jt % 2 == 0:
                        nc.vector.tensor_copy(hp[:, jt * 257 : jt * 257 + 256], ph[:, 0:256])
                    else:
                        nc.scalar.copy(hp[:, jt * 257 : jt * 257 + 256], ph[:, 0:256])
                    nc.scalar.activation(
                        sdst[:, jt : jt + 1], ph[:, 257:258],
                        mybir.ActivationFunctionType.Identity, bias=cv_sb[:, 1:2],
                    )
                    nc.vector.memset(hp[:, jt * 257 + 256 : jt * 257 + 257], 1.0)

                # ---- phase 1: masked attention weights + matmul accumulate ----
                # 4 key tiles per round: one 2 MB cast-DMA, then wide
                # stt/exp/mask ops amortize per-instruction overheads.
                po = [psp.tile([MB, 257], f32, name=f"po{ib}", tag="ps") for ib in range(8)]
                GR = 4
                groups = [list(range(g, min(g + GR, NJT))) for g in range(0, NJT, GR)]
                for jts in groups:
                    na = len(jts)
                    cw = na * W
                    j0 = jts[0]
                    adjT = ap_.tile([128, GR * W], f16, name="adjT", tag="adjT", bufs=3)
                    nc.gpsimd.dma_start(
                        adjT[:].rearrange("p (a w) -> p a w", w=W)[:, 0:na, :],
                        adjtd[j0 * 128 : (j0 + na) * 128, :].rearrange(
                            "(a p) w -> p a w", p=128
                        ),
                    )
                    e_t = ap_.tile([128, GR * W], f16, name="e_t", tag="e_t", bufs=2)
                    for t, jt in enumerate(jts):
                        nc.vector.tensor_scalar_add(
                            e_t[:, t * W : (t + 1) * W], sb1[:], sdst[:, jt : jt + 1]
                        )
                    l_t = ap_.tile([128, GR * W], f16, name="l_t", tag="l_t", bufs=2)
                    nc.vector.scalar_tensor_tensor(
                        l_t[:, 0:cw], e_t[:, 0:cw], NEG_SLOPE, e_t[:, 0:cw],
                        mybir.AluOpType.mult, mybir.AluOpType.max,
                    )
                    u_t = ap_.tile([128, GR * W], f16, name="u_t", tag="u_t", bufs=3)
                    nc.scalar.activation(
                        u_t[:, 0:cw], l_t[:, 0:cw], mybir.ActivationFunctionType.Exp
                    )
                    p_t = ap_.tile([128, GR * W], f16, name="p_t", tag="p_t", bufs=3)
                    import os as _os
                    if _os.environ.get("GAT_MASK_SPLIT") == "1":
                        h = (cw // 2) // W * W or W
                        nc.vector.tensor_mul(p_t[:, 0:h], adjT[:, 0:h], u_t[:, 0:h])
                        nc.gpsimd.tensor_mul(p_t[:, h:cw], adjT[:, h:cw], u_t[:, h:cw])
                    else:
                        nc.gpsimd.tensor_mul(p_t[:, 0:cw], adjT[:, 0:cw], u_t[:, 0:cw])
                    for t, jt in enumerate(jts):
                        for ib in range(8):
                            nc.tensor.matmul(
                                po[ib][:],
                                p_t[:, t * W + ib * MB : t * W + (ib + 1) * MB],
                                hp[:, jt * 257 : (jt + 1) * 257],
                                start=(jt == 0),
                                stop=(jt == NJT - 1),
                            )

                # ---- phase 2: normalize + store ----
                for ib in range(8):
                    r = fin.tile([MB, 1], f32, name="rcol", tag="rcol")
                    nc.vector.reciprocal(r[:], po[ib][:, 256:257])
                    ob = fin.tile([MB, F], f32, name="ob", tag="ob")
                    nc.vector.scalar_tensor_tensor(
                        ob[:], po[ib][:, 0:F], r[:], wb_sb[0:MB, :],
                        mybir.AluOpType.mult, mybir.AluOpType.add,
                    )
                    nc.sync.dma_start(outd[ib * MB : (ib + 1) * MB, :], ob[:])

    nc.compile()
    return _make_runner(nc, 8)


def _make_runner(nc, n_cores):
    """Compile-once sharded PJRT runner for the 8-core axon path."""
    import time
    import jax
    from jax.sharding import Mesh, PartitionSpec
    from jax.experimental.shard_map import shard_map
    from concourse import mybir
    from concourse.bass2jax import (
        _bass_exec_p,
        install_neuronx_cc_hook,
        partition_id_tensor,
    )

    install_neuronx_cc_hook()
    partition_name = nc.partition_id_tensor.name if nc.partition_id_tensor else None
    in_names, out_names, out_avals, zero_outs = [], [], [], []
    for alloc in nc.m.functions[0].allocations:
        if not isinstance(alloc, mybir.MemoryLocationSet):
            continue
        name = alloc.memorylocations[0].name
        if alloc.kind == "ExternalInput":
            if name != partition_name:
                in_names.append(name)
        elif alloc.kind == "ExternalOutput":
            out_names.append(name)
            shape = tuple(alloc.tensor_shape)
            dtype = mybir.dt.np(alloc.dtype)
            out_avals.append(jax.core.ShapedArray(shape, dtype))
            zero_outs.append(np.zeros(shape, dtype))
    n_params = len(in_names)
    all_in = in_names + out_names + ([partition_name] if partition_name else [])

    def _body(*args):
        operands = list(args)
        if partition_name is not None:
            operands.append(partition_id_tensor())
        return tuple(
            _bass_exec_p.bind(
                *operands,
                out_avals=tuple(out_avals),
                in_names=tuple(all_in),
                out_names=tuple(out_names),
                lowering_input_output_aliases=(),
                sim_require_finite=True,
                sim_require_nnan=True,
                nc=nc,
            )
        )

    devices = jax.devices()[:n_cores]
    mesh = Mesh(np.asarray(devices), ("core",))
    fn = jax.jit(
        shard_map(
            _body,
            mesh=mesh,
            in_specs=(PartitionSpec("core"),) * (n_params + len(out_names)),
            out_specs=(PartitionSpec("core"),) * len(out_names),
            check_rep=False,
        ),
        keep_unused=True,
    )

    def run(in_maps, iters=0):
        per_core = [[np.asarray(m[n]) for n in in_names] for m in in_maps]
        concat_in = [
            np.concatenate([per_core[c][i] for c in range(n_cores)], axis=0)
            for i in range(n_params)
        ]
        concat_zeros = [
            np.zeros((n_cores * z.shape[0], *z.shape[1:]), z.dtype) for z in zero_outs
        ]
        args = [jax.device_put(a) for a in concat_in + concat_zeros]
        out = fn(*args)
        jax.block_until_ready(out)
        times = []
        for _ in range(iters):
            t0 = time.perf_counter()
            out = fn(*args)
            jax.block_until_ready(out)
            times.append(time.perf_counter() - t0)
        results = [
            {
                name: np.asarray(out[i]).reshape(n_cores, *out_avals[i].shape)[c]
                for i, name in enumerate(out_names)
            }
            for c in range(n_cores)
        ]
        return results, (min(times) if times else None)

    return run


def kernel(node_embeddings, adj_matrix, W_w, W_b, a_src, a_dst, a_b):
    global _RUNNER, _last_in_maps
    if _RUNNER is None:
        _RUNNER = _build()

    X = np.asarray(node_embeddings, np.float32)
    adj = np.asarray(adj_matrix, np.int32)
    W_w = np.asarray(W_w, np.float32)
    W_b = np.asarray(W_b, np.float32)
    a_src = np.asarray(a_src, np.float32)
    a_dst = np.asarray(a_dst, np.float32)
    a_b = float(np.asarray(a_b))

    Xb = X.astype(ml_dtypes.bfloat16)
    xtb = np.zeros((F, NP), ml_dtypes.bfloat16)
    xtb[:, :N] = Xb.T
    g_src = (W_w.T @ a_src).astype(np.float32)
    g_dst = (W_w.T @ a_dst).astype(np.float32)
    wg = np.concatenate([W_w.T, g_src[:, None], g_dst[:, None]], axis=1)
    wg = np.ascontiguousarray(wg).astype(ml_dtypes.bfloat16)
    wbt = np.ascontiguousarray(np.tile(W_b, (128, 1))).astype(np.float16)
    cv = np.tile(
        np.array([[float(W_b @ a_src + a_b), float(W_b @ a_dst)]], np.float32),
        (128, 1),
    )

    in_maps = []
    for c in range(8):
        r0 = c * W
        rows = min(W, N - r0)
        adjt = np.zeros((NP, W), np.int32)
        adjt[:N, :rows] = adj[r0 : r0 + rows, :].T
        xwtb = np.zeros((F, W), ml_dtypes.bfloat16)
        xwtb[:, :rows] = Xb[r0 : r0 + rows].T
        in_maps.append(
            {"adjt": adjt, "xtb": xtb, "xwtb": xwtb, "wg": wg, "wbt": wbt, "cv": cv}
        )

    _last_in_maps = in_maps
    results, _ = _RUNNER(in_maps, iters=0)
    out = np.empty((N, F), np.float32)
    for c in range(8):
        r0 = c * W
        rows = min(W, N - r0)
        out[r0 : r0 + rows] = results[c]["out"][:rows]
    return out



# revision 2
# speedup vs baseline: 2.2177x; 2.2177x over previous
"""GAT layer kernel for 8 Trainium2 NeuronCores.

Strategy (row-sharded attention, per the sharding hint):
  - Core c owns query rows [c*1024, (c+1)*1024) of the 8000-node graph
    (1024 = 8*128; core 7's slice is host-padded; the key axis is padded
    to 8064 = 63*128 with masked-off columns).
  - Adjacency is uploaded as an int8 LOG-MASK in transposed+interleaved
    layout adjm[p, jt*W+q] = 0 if edge(query q, key jt*128+p) else -124.
    The mask is applied by ADDING it into the scores before leakyrelu:
    lrelu(-124 + e0) <= -23, and exp(-23) underflows to exactly 0 in
    fp16 -- identical to the reference's mask->-inf->lrelu->softmax
    weights.  This removes the separate mask-multiply pass entirely and
    cuts adjacency HBM traffic 4x vs int32.
  - H' = [W(x)|1] key tiles and the raw score projections s_src/s_dst
    are computed on device from a host-transposed bf16 X and a host-
    folded weight bundle wg = [W_w^T | g_src | g_dst].  All additive
    constants (W_b@a_src + W_b@a_dst + a_b) fold into the s_src row.
  - Scores in transposed layout (keys on partitions): ONE fused DVE
    scalar_tensor_tensor builds masked scores e = (mask + s_dst[j]) +
    s_src[i]; a second stt does leakyrelu as (0.2*e) max e; exp on
    ScalarE.  attn_unnorm @ [H | 1] accumulates into 8 PSUM banks;
    column 256 is the softmax denominator, applied as a reciprocal
    multiply (plus the W_b add) while draining PSUM.
"""
import os
import sys

sys.path.insert(0, "/opt/trn_rl_repo")

import numpy as np
import ml_dtypes

N, F = 8000, 256
NP = 8064          # padded key count (63 * 128)
W = 1024           # query rows per core (8 * 128; last core partly padding)
NJT = NP // 128    # 63 key tiles
MB = 128           # 128-row output blocks (full-width weights enable FWL)
NEG_SLOPE = 0.2
MASKVAL = -124.0   # log-mask: lrelu(-124+e0) <= -23 -> exp underflows to 0 in fp16

_RUNNER = None
_last_in_maps = None


def _build(repeat=1):
    import concourse.bass as bass
    import concourse.tile as tile
    from concourse import bacc, mybir

    f16 = mybir.dt.float16
    f32 = mybir.dt.float32
    bf16 = mybir.dt.bfloat16
    i8 = mybir.dt.int8

    GR = int(os.environ.get("GAT_GR", "4"))        # key tiles per group
    NG = (NJT + GR - 1) // GR
    LP = int(os.environ.get("GAT_LRELU_POOL", "0"))  # groups of lrelu on gpsimd
    HPC = int(os.environ.get("GAT_HPC", "0"))      # hp copies: 0 alt, 1 dve, 2 act
    pool_lrelu = set()
    if LP > 0:
        pool_lrelu = {min(NG - 1, round(i * NG / LP)) for i in range(LP)}

    nc = bacc.Bacc()
    adjtd = nc.dram_tensor("adjm", (128, NJT * W), i8, kind="ExternalInput")
    xtd = nc.dram_tensor("xtb", (F, NP), bf16, kind="ExternalInput")
    xwtd = nc.dram_tensor("xwtb", (F, W), bf16, kind="ExternalInput")
    wgd = nc.dram_tensor("wg", (F, 258), bf16, kind="ExternalInput")
    wbd = nc.dram_tensor("wbt", (128, F), f16, kind="ExternalInput")
    cvd = nc.dram_tensor("cv", (128, 1), f32, kind="ExternalInput")
    outd = nc.dram_tensor("out", (W, F), f32, kind="ExternalOutput")

    with tile.TileContext(nc) as tc:
        with (
            tc.tile_pool(name="pp", bufs=1) as pp,
            tc.tile_pool(name="att", bufs=2) as ap_,
            tc.tile_pool(name="fin", bufs=2) as fin,
            tc.tile_pool(name="ps", bufs=8, space="PSUM") as psp,
        ):
            for _rep in range(repeat):
                # ---- phase 0a: parameters and transposed activations ----
                wg_sb = [pp.tile([128, 258], bf16, name=f"wg{k}", tag=f"wg{k}") for k in range(2)]
                xt = [pp.tile([128, NP], bf16, name=f"xt{k}", tag=f"xt{k}") for k in range(2)]
                xwt = [pp.tile([128, W], bf16, name=f"xwt{k}", tag=f"xwt{k}") for k in range(2)]
                for k in range(2):
                    nc.sync.dma_start(wg_sb[k][:], wgd[k * 128 : (k + 1) * 128, :])
                    nc.sync.dma_start(xt[k][:], xtd[k * 128 : (k + 1) * 128, :])
                    nc.sync.dma_start(xwt[k][:], xwtd[k * 128 : (k + 1) * 128, :])
                wb_sb = pp.tile([128, F], f16, tag="wb")
                nc.sync.dma_start(wb_sb[:], wbd[:])
                cv_sb = pp.tile([128, 1], f32, tag="cv")
                nc.sync.dma_start(cv_sb[:], cvd[:])

                # ---- phase 0b: s_src row (this core's queries), consts folded --
                ssrc_row = pp.tile([1, W], f32, tag="ssrc")
                for ch in range(2):
                    ps = psp.tile([2, 512], f32, name="ps_s", tag="ps")
                    for k in range(2):
                        nc.tensor.matmul(
                            ps[:],
                            wg_sb[k][:, 256:258],
                            xwt[k][:, ch * 512 : (ch + 1) * 512],
                            start=(k == 0),
                            stop=(k == 1),
                        )
                    nc.vector.tensor_scalar_add(
                        ssrc_row[0:1, ch * 512 : (ch + 1) * 512], ps[0:1, :], cv_sb[0:1, 0:1]
                    )

                # ---- phase 0c: broadcast s_src across partitions via PE ----
                ones1 = pp.tile([1, 128], f32, tag="ones1")
                nc.vector.memset(ones1[:], 1.0)
                sb1 = pp.tile([128, W], f16, tag="sb1")
                for ch in range(2):
                    psb_t = psp.tile([128, 512], f32, name="ps_b", tag="ps")
                    nc.tensor.matmul(
                        psb_t[:], ones1[:], ssrc_row[0:1, ch * 512 : (ch + 1) * 512],
                        start=True, stop=True,
                    )
                    nc.vector.tensor_copy(sb1[:, ch * 512 : (ch + 1) * 512], psb_t[:])

                # ---- phase 0d: H' key tiles + raw s_dst columns (per group) ----
                hp_g = []
                sd_g = []
                for g in range(NG):
                    na = min(GR, NJT - g * GR)
                    hp_g.append(pp.tile([128, na * 257], f16, name=f"hp{g}", tag=f"hp{g}"))
                    sd_g.append(pp.tile([128, GR], f32, name=f"sd{g}", tag=f"sd{g}"))
                for jt in range(NJT):
                    g, t = jt // GR, jt % GR
                    ph = psp.tile([128, 258], f32, name="ps_h", tag="ps")
                    for k in range(2):
                        nc.tensor.matmul(
                            ph[:],
                            xt[k][:, jt * 128 : (jt + 1) * 128],
                            wg_sb[k][:],
                            start=(k == 0),
                            stop=(k == 1),
                        )
                    use_dve = (jt % 2 == 0) if HPC == 0 else (HPC == 1)
                    if use_dve:
                        nc.vector.tensor_copy(hp_g[g][:, t * 257 : t * 257 + 256], ph[:, 0:256])
                    else:
                        nc.scalar.copy(hp_g[g][:, t * 257 : t * 257 + 256], ph[:, 0:256])
                    nc.vector.tensor_copy(sd_g[g][:, t : t + 1], ph[:, 257:258])
                    nc.vector.memset(hp_g[g][:, t * 257 + 256 : t * 257 + 257], 1.0)

                # ---- phase 1: masked scores -> lrelu -> exp -> matmul accum ----
                po = [psp.tile([MB, 257], f32, name=f"po{ib}", tag="ps") for ib in range(8)]
                for g in range(NG):
                    na = min(GR, NJT - g * GR)
                    cw = na * W
                    adjT = ap_.tile([128, GR * W], f16, name="adjT", tag="adjT", bufs=3)
                    nc.gpsimd.dma_start(
                        adjT[:, 0:cw], adjtd[:, g * GR * W : g * GR * W + cw]
                    )
                    e_t = ap_.tile([128, GR * W], f16, name="e_t", tag="e_t", bufs=2)
                    for t in range(na):
                        nc.vector.scalar_tensor_tensor(
                            e_t[:, t * W : (t + 1) * W],
                            adjT[:, t * W : (t + 1) * W],
                            sd_g[g][:, t : t + 1],
                            sb1[:],
                            mybir.AluOpType.add,
                            mybir.AluOpType.add,
                        )
                    l_t = ap_.tile([128, GR * W], f16, name="l_t", tag="l_t", bufs=2)
                    eng = nc.gpsimd if g in pool_lrelu else nc.vector
                    eng.scalar_tensor_tensor(
                        l_t[:, 0:cw], e_t[:, 0:cw], NEG_SLOPE, e_t[:, 0:cw],
                        mybir.AluOpType.mult, mybir.AluOpType.max,
                    )
                    u_t = ap_.tile([128, GR * W], f16, name="u_t", tag="u_t", bufs=2)
                    nc.scalar.activation(
                        u_t[:, 0:cw], l_t[:, 0:cw], mybir.ActivationFunctionType.Exp
                    )
                    for t in range(na):
                        jt = g * GR + t
                        for ib in range(8):
                            nc.tensor.matmul(
                                po[ib][:],
                                u_t[:, t * W + ib * MB : t * W + (ib + 1) * MB],
                                hp_g[g][:, t * 257 : (t + 1) * 257],
                                start=(jt == 0),
                                stop=(jt == NJT - 1),
                            )

                # ---- phase 2: normalize + store ----
                for ib in range(8):
                    r = fin.tile([MB, 1], f32, name="rcol", tag="rcol")
                    nc.vector.reciprocal(r[:], po[ib][:, 256:257])
                    ob = fin.tile([MB, F], f32, name="ob", tag="ob")
                    nc.vector.scalar_tensor_tensor(
                        ob[:], po[ib][:, 0:F], r[:], wb_sb[0:MB, :],
                        mybir.AluOpType.mult, mybir.AluOpType.add,
                    )
                    nc.sync.dma_start(outd[ib * MB : (ib + 1) * MB, :], ob[:])

    nc.compile()
    return _make_runner(nc, 8)


def _make_runner(nc, n_cores):
    """Compile-once sharded PJRT runner for the 8-core axon path."""
    import time
    import jax
    from jax.sharding import Mesh, PartitionSpec
    from jax.experimental.shard_map import shard_map
    from concourse import mybir
    from concourse.bass2jax import (
        _bass_exec_p,
        install_neuronx_cc_hook,
        partition_id_tensor,
    )

    install_neuronx_cc_hook()
    partition_name = nc.partition_id_tensor.name if nc.partition_id_tensor else None
    in_names, out_names, out_avals, zero_outs = [], [], [], []
    for alloc in nc.m.functions[0].allocations:
        if not isinstance(alloc, mybir.MemoryLocationSet):
            continue
        name = alloc.memorylocations[0].name
        if alloc.kind == "ExternalInput":
            if name != partition_name:
                in_names.append(name)
        elif alloc.kind == "ExternalOutput":
            out_names.append(name)
            shape = tuple(alloc.tensor_shape)
            dtype = mybir.dt.np(alloc.dtype)
            out_avals.append(jax.core.ShapedArray(shape, dtype))
            zero_outs.append(np.zeros(shape, dtype))
    n_params = len(in_names)
    all_in = in_names + out_names + ([partition_name] if partition_name else [])

    def _body(*args):
        operands = list(args)
        if partition_name is not None:
            operands.append(partition_id_tensor())
        return tuple(
            _bass_exec_p.bind(
                *operands,
                out_avals=tuple(out_avals),
                in_names=tuple(all_in),
                out_names=tuple(out_names),
                lowering_input_output_aliases=(),
                sim_require_finite=True,
                sim_require_nnan=True,
                nc=nc,
            )
        )

    devices = jax.devices()[:n_cores]
    mesh = Mesh(np.asarray(devices), ("core",))
    fn = jax.jit(
        shard_map(
            _body,
            mesh=mesh,
            in_specs=(PartitionSpec("core"),) * (n_params + len(out_names)),
            out_specs=(PartitionSpec("core"),) * len(out_names),
            check_rep=False,
        ),
        keep_unused=True,
    )

    def run(in_maps, iters=0):
        per_core = [[np.asarray(m[n]) for n in in_names] for m in in_maps]
        concat_in = [
            np.concatenate([per_core[c][i] for c in range(n_cores)], axis=0)
            for i in range(n_params)
        ]
        concat_zeros = [
            np.zeros((n_cores * z.shape[0], *z.shape[1:]), z.dtype) for z in zero_outs
        ]
        args = [jax.device_put(a) for a in concat_in + concat_zeros]
        out = fn(*args)
        jax.block_until_ready(out)
        times = []
        for _ in range(iters):
            t0 = time.perf_counter()
            out = fn(*args)
            jax.block_until_ready(out)
            times.append(time.perf_counter() - t0)
        results = [
            {
                name: np.asarray(out[i]).reshape(n_cores, *out_avals[i].shape)[c]
                for i, name in enumerate(out_names)
            }
            for c in range(n_cores)
        ]
        return results, (min(times) if times else None)

    return run


def kernel(node_embeddings, adj_matrix, W_w, W_b, a_src, a_dst, a_b):
    global _RUNNER, _last_in_maps
    if _RUNNER is None:
        _RUNNER = _build()

    X = np.asarray(node_embeddings, np.float32)
    adj = np.asarray(adj_matrix, np.int32)
    W_w = np.asarray(W_w, np.float32)
    W_b = np.asarray(W_b, np.float32)
    a_src = np.asarray(a_src, np.float32)
    a_dst = np.asarray(a_dst, np.float32)
    a_b = float(np.asarray(a_b))

    Xb = X.astype(ml_dtypes.bfloat16)
    xtb = np.zeros((F, NP), ml_dtypes.bfloat16)
    xtb[:, :N] = Xb.T
    g_src = (W_w.T @ a_src).astype(np.float32)
    g_dst = (W_w.T @ a_dst).astype(np.float32)
    wg = np.concatenate([W_w.T, g_src[:, None], g_dst[:, None]], axis=1)
    wg = np.ascontiguousarray(wg).astype(ml_dtypes.bfloat16)
    wbt = np.ascontiguousarray(np.tile(W_b, (128, 1))).astype(np.float16)
    cv = np.full((128, 1), float(W_b @ a_src + W_b @ a_dst + a_b), np.float32)

    in_maps = []
    for c in range(8):
        r0 = c * W
        rows = min(W, N - r0)
        T = np.full((NP, W), MASKVAL, np.int8)
        T[:N, :rows] = np.where(adj[r0 : r0 + rows, :].T != 0, 0, MASKVAL).astype(
            np.int8
        )
        adjm = np.ascontiguousarray(
            T.reshape(NJT, 128, W).transpose(1, 0, 2).reshape(128, NJT * W)
        )
        xwtb = np.zeros((F, W), ml_dtypes.bfloat16)
        xwtb[:, :rows] = Xb[r0 : r0 + rows].T
        in_maps.append(
            {"adjm": adjm, "xtb": xtb, "xwtb": xwtb, "wg": wg, "wbt": wbt, "cv": cv}
        )

    _last_in_maps = in_maps
    results, _ = _RUNNER(in_maps, iters=0)
    out = np.empty((N, F), np.float32)
    for c in range(8):
        r0 = c * W
        rows = min(W, N - r0)
        out[r0 : r0 + rows] = results[c]["out"][:rows]
    return out
